# revision 3
# baseline (speedup 1.0000x reference)
"""Bass/Trainium2 kernel for nn_HWNNLayer (gnn_message_passing).

Computes out = wavelets @ diag(d) @ wavelets_inv @ features @ W  on 8 cores.

Sharding (hardcoded, 8 cores):
  - wavelets_inv row-sharded: core j computes y_j = Winv[rows_j,:] @ x  (rows_j = 2048 rows)
  - diag applied to y_j rows
  - wavelets column-sharded with the SAME index block: core j computes the
    full-size partial  out_j = Wv[:, rows_j] @ y_j ; host sums the 8 partials.
  - features / W replicated; x = features @ W computed on every core.

Device layout: all matmuls run "transposed" so the big matrices stream as the
moving operand in natural row-major order:
  yT_j  [32,2048]  = x.T @ winvT_j          (winvT_j = Winv[rows_j,:].T, host-transposed)
  outT_j[32,16384] = y'_j.T @ wvT_j         (wvT_j = wavelets.T[rows_j,:], host-transposed)
The tiny [128,32] x / y' tiles are the stationary operand.

Precision scheme (rel-err budget 2e-2, measured end-to-end 1.67e-2):
  - winvT streams fully as float8e3 (1-3-4);
  - wvT streams half float8e3, half float16 (contraction split per core);
  - x / y' stationary tiles are float16 (mixed-dtype matmul verified exact
    on HW); featT/W in bfloat16 for mm0.
Rationale: fp8e4's 3-bit mantissa fails the error budget (3.8e-2); e3m4 on
everything is 1.91e-2 (4.6% margin). The asymmetric split keeps the same
error as 3/4-fp8-everywhere but puts the DMA slack in phase 1, where the mt
(wvT) stream prefetches behind mm1's PE-bound tail. PE floor: F=32 uses 32
of 128 PE columns -> 1 cycle/moving-column: 2*512 matmuls x 512 cols
~ 242 us measured; total DMA 80 MiB ~ 185 us at the measured 434 GB/s.

DMA layout: big-matrix chunks pack 4 (fp8) / 2 (fp16) consecutive source
rows per partition ("(p t) r -> p t r"), giving 8 KiB contiguous per-
partition lines (434 GB/s vs 302 GB/s at 2 KiB lines). The matmul contracts
over partitions symmetrically, so both operands just need the SAME row
permutation: mm0 and the yT transposes read their stationary operand with a
stride-4 (or -2) column AP, producing x / y' directly in the packed order;
diag is host-packed to match.

Sync-wait budget (walrus ISA limits): matmuls lower with ONE sync-wait slot;
HWDGE DMAs have two. Mechanisms used to stay inside that:
  - "observer" matmuls (obs_ps scratch) advance the PE clock past DVE/DMA
    ticks so real matmuls only wait on the DMA they stream from;
  - "bank-claim" matmuls absorb the PSUM bank-transition wait when a pool
    recycles banks between phases/groups;
  - small/aux DMAs ride SWDGE (gpsimd); _split_excess_waits moves any
    remaining excess onto standalone EventSemaphore instructions.
"""

import numpy as np
import ml_dtypes

from concourse import bass, mybir, tile
from concourse.bass_utils import run_bass_kernel_spmd
from concourse.masks import make_identity
from concourse.tile import add_dep_helper

N = 16384
F = 32
NCORES = 8
S = N // NCORES  # rows per core = 2048

K81 = 1024       # wvT contraction rows (of 2048) streamed as fp8e3; rest fp16

DT = mybir.dt.float32
DT8 = mybir.dt.float8e3
DT16 = mybir.dt.float16
DTB = mybir.dt.bfloat16
NP8 = ml_dtypes.float8_e3m4
NP16 = np.float16
NPB = ml_dtypes.bfloat16

# packed y_sb column order: j<8 -> fp8 rows (chunks of 512, 4 rows/partition);
# j>=8 -> fp16 rows (chunks of 256, 2 rows/partition)
Y8CH = K81 // 512        # 2 fp8 y chunks
Y16CH = (S - K81) // 256  # 4 fp16 y chunks


def build_bass(n=N, s=S, reps=1):
    """Build the single-core Bass program (SPMD: same NEFF on all cores).

    reps > 1 repeats the whole compute body inside one NEFF (timing aid:
    per-iteration device time = slope of wall time vs reps, which cancels
    the ~100 ms axon dispatch overhead)."""
    nc = bass.Bass()

    featT = nc.dram_tensor("featT", [F, n], DTB, kind="ExternalInput")
    w = nc.dram_tensor("w", [F, F], DT, kind="ExternalInput")
    winvT8 = nc.dram_tensor("winvT8", [n, s], DT8, kind="ExternalInput")
    wvT8 = nc.dram_tensor("wvT8", [K81, n], DT8, kind="ExternalInput")
    wvT16 = nc.dram_tensor("wvT16", [s - K81, n], DT16, kind="ExternalInput")
    diag = nc.dram_tensor("diag", [128, s // 128], DT, kind="ExternalInput")
    outT = nc.dram_tensor("outT", [F, n], DT, kind="ExternalOutput")
    chk = nc.dram_tensor("chk", [F, 512], DT, kind="ExternalOutput")

    CC = n // 512      # 32 winvT packed chunks (512 rows, 4/partition)
    RB = s // 512      # yT 512-col chunks (psum banks live in mm1)
    NG = n // 2048     # output column groups for mm2 (4 psum banks each)
    XCH = CC           # x packed chunks (512 rows each)
    FTC = 4096         # featT chunk width (4 SWDGE DMAs)

    with tile.TileContext(nc) as tc:
        with (
            tc.tile_pool(name="const", bufs=1) as constp,
            tc.tile_pool(name="xsb", bufs=1) as xsbp,
            tc.tile_pool(name="ysb", bufs=1) as ysbp,
            tc.tile_pool(name="ft", bufs=2) as ftp,
            tc.tile_pool(name="wt", bufs=6) as wtp,
            tc.tile_pool(name="mt8", bufs=4) as mtp8,
            tc.tile_pool(name="mt16", bufs=4) as mtp16,
            tc.tile_pool(name="ot", bufs=2) as otp,
            tc.tile_pool(name="obs", bufs=1, space="PSUM") as obsp,
        ):
            w_sb = constp.tile([F, F], DT)
            nc.gpsimd.dma_start(w_sb[:], w[:])
            diag_sb = constp.tile([128, s // 128], DT)
            nc.gpsimd.dma_start(diag_sb[:], diag[:])
            id_sb = constp.tile([F, F], DT)
            make_identity(nc, id_sb[:])
            # bf16 copy of W for mm0 (moving operand)
            w16_sb = constp.tile([F, F], DTB)
            nc.vector.tensor_copy(w16_sb[:], w_sb[:])
            # DVE observer: one DVE op sees the diag DMA so later
            # tensor_scalar_muls only wait on their PE transpose.
            dvescr = constp.tile([128, s // 128], DT)
            nc.vector.tensor_copy(dvescr[:], diag_sb[:])

            # scratch PSUM bank the observer matmuls write into (one 32-col
            # slice each so nothing is ever dead-stored).
            obs_ps = obsp.tile([F, 512], DT)
            obs_n = [0]
            last_ob = [None]

            def observe(ap):
                """PE matmul reading `ap` ([P,32] or [32,32] slice): advances
                the PE clock past ap's producer with a single wait."""
                sl = obs_ps[:, (obs_n[0] % 16) * F:(obs_n[0] % 16 + 1) * F]
                obs_n[0] += 1
                ob = nc.tensor.matmul(sl, ap, ap, start=True, stop=True)
                last_ob[0] = ob
                return ob

            def order_after_ob(mm):
                """Force the scheduler to keep `mm` after the latest observer
                so cross-engine waits land on the observer, keeping `mm` at a
                single sync wait."""
                if last_ob[0] is not None:
                    add_dep_helper(mm.ins, last_ob[0].ins, sync=False,
                                   reason="order after observer")

            # packed layouts: x_sb col group j = cc*4+t holds x row 512cc+4p+t
            # at partition p; y_sb col group j (see Y8CH/Y16CH) likewise.
            x_sb = xsbp.tile([128, XCH * 4 * F], DT16)   # [128, 4096]
            yT_sb = ysbp.tile([F, s], DT)                # y.T, [32, 2048]
            y_sb = ysbp.tile([128, (s // 128) * F], DT16)  # [128, 512]

            observe(w_sb[:])
            observe(id_sb[:])

            for _rep in range(reps):
                # ---- mm0: x = features @ W, written in packed row order via
                # stride-4 stationary slices.
                with tc.tile_pool(name="ps_x", bufs=2, space="PSUM") as ps_x:
                    for fb in range(n // FTC):
                        ft = ftp.tile([F, FTC], DTB, tag="ft")
                        nc.gpsimd.dma_start(ft[:], featT[:, fb * FTC:(fb + 1) * FTC])
                        ftv = ft[:].rearrange("f (c p t) -> f c t p", t=4, p=128)
                        for c in range(FTC // 512):
                            cc = fb * (FTC // 512) + c
                            for t in range(4):
                                ps = ps_x.tile([128, F], DT)
                                mm = nc.tensor.matmul(
                                    ps[:], ftv[:, c, t, :], w16_sb[:],
                                    start=True, stop=True,
                                )
                                if c == 0 and t == 0:
                                    order_after_ob(mm)
                                nc.vector.tensor_copy(
                                    x_sb[:, (cc * 4 + t) * F:(cc * 4 + t + 1) * F],
                                    ps[:])
                        # PE sees this group's DVE evacuations so the next group's
                        # matmuls only wait on their featT DMA.
                        jl = (fb * (FTC // 512) + (FTC // 512) - 1) * 4 + 3
                        observe(x_sb[:, jl * F:(jl + 1) * F])

                # ---- mm1: yT = x.T @ winvT  ([32, s] accumulated over 128 steps)
                with tc.tile_pool(name="ps_y", bufs=RB, space="PSUM") as ps_y:
                    yps = [ps_y.tile([F, 512], DT, name="yps", tag="yps")
                           for _ in range(RB)]
                    last_cl = None
                    for rb in range(RB):
                        # bank-claim: absorbs the PSUM bank-transition wait so the
                        # first accumulating matmul only waits on its DMA
                        cl = nc.tensor.matmul(yps[rb][:, 0:F], w_sb[:], w_sb[:],
                                              start=True, stop=True)
                        order_after_ob(cl)
                        last_cl = cl
                    for cc in range(CC):  # 512-row packed chunks (512 KiB each)
                        wt = wtp.tile([128, 4, s], DT8, tag="wt8")
                        nc.sync.dma_start(
                            wt[:], winvT8[cc * 512:(cc + 1) * 512, :].rearrange(
                                "(p t) r -> p t r", p=128))
                        for t in range(4):
                            for rb in range(RB):
                                mm = nc.tensor.matmul(
                                    yps[rb][:],
                                    x_sb[:, (cc * 4 + t) * F:(cc * 4 + t + 1) * F],
                                    wt[:, t, rb * 512:(rb + 1) * 512],
                                    start=(cc == 0 and t == 0),
                                    stop=(cc == CC - 1 and t == 3),
                                )
                                if cc == 0 and t == 0 and rb == 0:
                                    add_dep_helper(mm.ins, last_cl.ins, sync=False,
                                                   reason="order after bank claims")
                    for rb in range(RB):
                        nc.vector.tensor_copy(yT_sb[:, rb * 512:(rb + 1) * 512],
                                              yps[rb][:])

                # ---- transpose yT -> packed y tiles [128, 32], scaled by diag.
                # fp8 range (rows < K81): stride-4 column slices; fp16 range:
                # stride-2. diag_sb is host-packed in the same column order.
                with tc.tile_pool(name="ps_t", bufs=2, space="PSUM") as ps_t:
                    observe(yT_sb[:, s - F:s])
                    pts = [ps_t.tile([128, F], DT, name="pt", tag="pt")
                           for _ in range(2)]
                    for i, pt in enumerate(pts):
                        cl = nc.tensor.matmul(pt[0:F, 0:F], w_sb[:], w_sb[:],
                                              start=True, stop=True)
                        order_after_ob(cl)
                    yTv8 = yT_sb[:, 0:K81].rearrange("f (c p t) -> f c t p",
                                                     t=4, p=128)
                    yTv16 = yT_sb[:, K81:s].rearrange("f (c p t) -> f c t p",
                                                      t=2, p=128)
                    ji = 0
                    for c in range(Y8CH):
                        for t in range(4):
                            pt = pts[ji % 2]
                            nc.tensor.transpose(pt[:], yTv8[:, c, t, :], id_sb[:])
                            nc.vector.tensor_scalar_mul(
                                y_sb[:, ji * F:(ji + 1) * F], pt[:],
                                diag_sb[:, ji:ji + 1])
                            ji += 1
                    for c in range(Y16CH):
                        for t in range(2):
                            pt = pts[ji % 2]
                            nc.tensor.transpose(pt[:], yTv16[:, c, t, :], id_sb[:])
                            nc.vector.tensor_scalar_mul(
                                y_sb[:, ji * F:(ji + 1) * F], pt[:],
                                diag_sb[:, ji:ji + 1])
                            ji += 1
                    observe(y_sb[:, (ji - 1) * F:ji * F])

                # ---- mm2: outT = y'.T @ wvT  ([32, n] in groups of 2048 cols)
                with tc.tile_pool(name="ps_o", bufs=4, space="PSUM") as ps_o:
                    for ng in range(NG):
                        ops = [ps_o.tile([F, 512], DT, name="ops", tag="ops")
                               for _ in range(4)]
                        last_cl = None
                        for nb in range(4):
                            cl = nc.tensor.matmul(ops[nb][:, 0:F], w_sb[:], w_sb[:],
                                                  start=True, stop=True)
                            order_after_ob(cl)
                            last_cl = cl
                        ki = 0  # contraction step (of 16) within the group
                        for c in range(Y8CH):  # fp8: 512-row packed chunks
                            mt = mtp8.tile([128, 4, 2048], DT8, tag="mt8")
                            nc.sync.dma_start(
                                mt[:], wvT8[c * 512:(c + 1) * 512,
                                            ng * 2048:(ng + 1) * 2048].rearrange(
                                    "(p t) r -> p t r", p=128))
                            for t in range(4):
                                j = c * 4 + t
                                for nb in range(4):
                                    mm = nc.tensor.matmul(
                                        ops[nb][:],
                                        y_sb[:, j * F:(j + 1) * F],
                                        mt[:, t, nb * 512:(nb + 1) * 512],
                                        start=(ki == 0), stop=(ki == 15),
                                    )
                                    if ki == 0 and nb == 0:
                                        add_dep_helper(
                                            mm.ins, last_cl.ins, sync=False,
                                            reason="order after bank claims")
                                ki += 1
                        for c in range(Y16CH):  # fp16: 256-row packed chunks
                            mt = mtp16.tile([128, 2, 2048], DT16, tag="mt16")
                            nc.sync.dma_start(
                                mt[:], wvT16[c * 256:(c + 1) * 256,
                                             ng * 2048:(ng + 1) * 2048].rearrange(
                                    "(p t) r -> p t r", p=128))
                            for t in range(2):
                                j = Y8CH * 4 + c * 2 + t
                                for nb in range(4):
                                    nc.tensor.matmul(
                                        ops[nb][:],
                                        y_sb[:, j * F:(j + 1) * F],
                                        mt[:, t, nb * 512:(nb + 1) * 512],
                                        start=(ki == 0), stop=(ki == 15),
                                    )
                                ki += 1
                        ot = otp.tile([F, 2048], DT, tag="ot")
                        for nb in range(4):
                            nc.vector.tensor_copy(
                                ot[:, nb * 512:(nb + 1) * 512], ops[nb][:])
                        nc.gpsimd.dma_start(outT[:, ng * 2048:(ng + 1) * 2048], ot[:])
                        # PE sees this group's evacuations before the next group
                        # recycles the same PSUM banks (read a slice of the LAST
                        # copy so its DVE tick dominates the whole group).
                        observe(ot[:, 3 * 512:3 * 512 + F])

            chk_sb = constp.tile([F, 512], DT)
            nc.vector.tensor_copy(chk_sb[:], obs_ps[:])
            nc.gpsimd.dma_start(chk[:], chk_sb[:])

    _split_excess_waits(nc)
    return nc


def _split_excess_waits(nc, limit=1):
    """Walrus allows a single sync-wait slot on fused fp32 matmuls and DMA
    triggers. Move any extra waits onto standalone EventSemaphore
    instructions inserted just before the offender in its engine stream
    (what raw-bass wait_ge would emit)."""
    nev = [0]
    for f in nc.m.functions:
        for b in f.blocks:
            out = []
            changed = False
            for inst in b.instructions:
                si = inst.sync_info
                waits = list(si.on_wait) if si is not None else []
                if len(waits) > limit:
                    changed = True
                    for wv in waits[:-limit]:
                        ev = mybir.InstEventSemaphore(
                            name=f"splitwait_{nev[0]}", engine=inst.engine,
                            ins=[], outs=[])
                        nev[0] += 1
                        ev.sync_info = mybir.SyncInfo(on_wait=[wv], on_update=[])
                        out.append(ev)
                    inst.sync_info = mybir.SyncInfo(
                        on_wait=waits[-limit:], on_update=list(si.on_update))
                out.append(inst)
            if changed:
                b.instructions = out
    return nc


def _cast_T(a, np_dt):
    """Cache-blocked cast + out-of-place transpose ([r,c] fp32 -> [c,r] np_dt)."""
    r, c = a.shape
    out = np.empty((c, r), dtype=np_dt)
    B = 512
    for i in range(0, r, B):
        for k in range(0, c, B):
            out[k:k + B, i:i + B] = a[i:i + B, k:k + B].astype(np_dt).T
    return out


def _pack_diag(d):
    """Host-pack the per-core diag slice [S] into the y_sb column order:
    col j<8: row (j//4)*512 + 4p + j%4 ; col j>=8: K81 + ((j-8)//2)*256 + 2p
    + (j-8)%2, for partition p."""
    out = np.empty((128, S // 128), dtype=np.float32)
    p = np.arange(128)
    j = 0
    for c in range(Y8CH):
        for t in range(4):
            out[:, j] = d[c * 512 + 4 * p + t]
            j += 1
    for c in range(Y16CH):
        for t in range(2):
            out[:, j] = d[K81 + c * 256 + 2 * p + t]
            j += 1
    return np.ascontiguousarray(out)


def _shard_inputs(features, wavelets, wavelets_inv, diag_filter, weight_matrix):
    from concurrent.futures import ThreadPoolExecutor
    featT = np.ascontiguousarray(features.T.astype(NPB))
    with ThreadPoolExecutor(max_workers=16) as ex:
        wvT8_parts = list(ex.map(
            lambda j: _cast_T(wavelets[:, j * S:j * S + K81], NP8),
            range(NCORES)))
        wvT16_parts = list(ex.map(
            lambda j: _cast_T(wavelets[:, j * S + K81:(j + 1) * S], NP16),
            range(NCORES)))
        winvT8_parts = list(ex.map(
            lambda j: _cast_T(wavelets_inv[j * S:(j + 1) * S, :], NP8),
            range(NCORES)))
    in_maps = []
    for j in range(NCORES):
        r0, r1 = j * S, (j + 1) * S
        in_maps.append({
            "featT": featT,
            "w": np.ascontiguousarray(weight_matrix),
            "winvT8": winvT8_parts[j],
            "wvT8": wvT8_parts[j],
            "wvT16": wvT16_parts[j],
            "diag": _pack_diag(diag_filter[r0:r1]),
        })
    return in_maps


def _run(inputs, trace=False, **trace_kwargs):
    in_maps = _shard_inputs(
        np.asarray(inputs["features"], dtype=np.float32),
        np.asarray(inputs["wavelets"], dtype=np.float32),
        np.asarray(inputs["wavelets_inv"], dtype=np.float32),
        np.asarray(inputs["diag_filter"], dtype=np.float32),
        np.asarray(inputs["weight_matrix"], dtype=np.float32),
    )
    nc = build_bass()
    res = run_bass_kernel_spmd(nc, in_maps, list(range(NCORES)), trace=trace,
                               **trace_kwargs)
    acc = np.zeros((F, N), dtype=np.float64)
    for j in range(NCORES):
        acc += res.results[j]["outT"]
    out = np.ascontiguousarray(acc.T.astype(np.float32))
    return out, res


def kernel(**inputs):
    out, _ = _run(inputs, trace=False)
    return out


def kernel_traced(**inputs):
    out, res = _run(inputs, trace=True)
    return out, res


# revision 8
# speedup vs baseline: 1.0437x; 1.0437x over previous
"""Bass/Trainium2 kernel for nn_HWNNLayer (gnn_message_passing).

Computes out = wavelets @ diag(d) @ wavelets_inv @ features @ W  on 8 cores.

Sharding (hardcoded, 8 cores):
  - wavelets_inv row-sharded: core j computes y_j = Winv[rows_j,:] @ x  (rows_j = 2048 rows)
  - diag applied to y_j rows
  - wavelets column-sharded with the SAME index block: core j computes the
    full-size partial  out_j = Wv[:, rows_j] @ y_j ; host sums the 8 partials.
  - features / W replicated; x = features @ W computed on every core.

Device layout: all matmuls run "transposed" so the big matrices stream as the
moving operand in natural row-major order:
  yT_j  [32,2048]  = x.T @ winvT_j          (winvT_j = Winv[rows_j,:].T, host-transposed)
  outT_j[32,16384] = y'_j.T @ wvT_j         (wvT_j = wavelets.T[rows_j,:], host-transposed)
The tiny [128,32] x / y' tiles are the stationary operand.

Precision scheme (rel-err budget 2e-2, measured end-to-end 1.67e-2): 3/4 of
each big matrix's contraction range streams as float8e3 (1-3-4), the rest as
float16; x / y' stationary tiles are float16 (mixed-dtype matmul verified
exact on HW); featT/W in bfloat16 for mm0. fp8e4's 3-bit mantissa fails the
budget (3.8e-2); fp8e3-everywhere is 1.91e-2 (4.6% margin - too thin).

Performance model (per core): PE is the floor. F=32 uses 32 of 128 PE
columns, so the moving stream costs 1 cycle/column: 2*512 matmuls x 512
cols ~ 121 us per mm phase (measured). DMA: 40 MiB per phase at the
measured 434 GB/s ~ 92 us, hidden under the PE phases; total 80 MiB.

DMA layout: big-matrix chunks pack 4 (fp8) / 2 (fp16) consecutive source
rows per partition ("(p t) r -> p t r"), giving 8 KiB contiguous per-
partition lines (434 GB/s vs 302 GB/s at 2 KiB lines). The matmul contracts
over partitions symmetrically, so both operands just need the SAME row
permutation: mm0 and the yT transposes read their stationary operand with a
stride-4 (or -2) column AP, producing x / y' directly in the packed order;
diag is host-packed to match.

PSUM: mm2 runs groups on alternating bank sets (2 sets x 4 banks) so a
group's DVE/Act evacuations overlap the next group's accumulation; the
inter-group observers write into the idle set, absorbing the WAR wait a
full group early. Evacuations alternate DVE / Activation engines.

Sync-wait budget (walrus ISA limits): matmuls lower with ONE sync-wait slot;
HWDGE DMAs have two. Mechanisms used to stay inside that:
  - "observer" matmuls advance the PE clock past DVE/Act/DMA ticks so real
    matmuls only wait on the DMA they stream from;
  - "bank-claim" matmuls absorb the PSUM bank-transition wait when a pool
    recycles banks between phases/groups;
  - tiny constant DMAs ride SWDGE (gpsimd); _split_excess_waits moves any
    remaining excess onto standalone EventSemaphore instructions.
"""

import numpy as np
import ml_dtypes

from concourse import bass, mybir, tile
from concourse.bass_utils import run_bass_kernel_spmd
from concourse.masks import make_identity
from concourse.tile import add_dep_helper

N = 16384
F = 32
NCORES = 8
S = N // NCORES  # rows per core = 2048

R81 = 12288      # winvT contraction rows (of 16384) streamed as fp8e3; rest fp16
K81 = 1536       # wvT contraction rows (of 2048) streamed as fp8e3; rest fp16

DT = mybir.dt.float32
DT8 = mybir.dt.float8e3
DT16 = mybir.dt.float16
DTB = mybir.dt.bfloat16
NP8 = ml_dtypes.float8_e3m4
NP16 = np.float16
NPB = ml_dtypes.bfloat16

# packed x_sb column order: fp8 rows (chunks of 512, 4 rows/partition) then
# fp16 rows (chunks of 256, 2 rows/partition); same for y_sb.
X8CH = R81 // 512          # 24 fp8 x chunks
X16CH = (N - R81) // 256   # 16 fp16 x chunks
Y8CH = K81 // 512          # 3 fp8 y chunks
Y16CH = (S - K81) // 256   # 2 fp16 y chunks


def build_bass(n=N, s=S, reps=1):
    """Build the single-core Bass program (SPMD: same NEFF on all cores).

    reps > 1 repeats the whole compute body inside one NEFF (timing aid:
    per-iteration device time = slope of wall time vs reps, which cancels
    the ~100 ms axon dispatch overhead)."""
    import os
    skip = set(os.environ.get("KSKIP", "").split(","))
    nc = bass.Bass()

    featT = nc.dram_tensor("featT", [F, n], DTB, kind="ExternalInput")
    w = nc.dram_tensor("w", [F, F], DT, kind="ExternalInput")
    winvT8 = nc.dram_tensor("winvT8", [R81, s], DT8, kind="ExternalInput")
    winvT16 = nc.dram_tensor("winvT16", [n - R81, s], DT16, kind="ExternalInput")
    wvT8 = nc.dram_tensor("wvT8", [K81, n], DT8, kind="ExternalInput")
    wvT16 = nc.dram_tensor("wvT16", [s - K81, n], DT16, kind="ExternalInput")
    diag = nc.dram_tensor("diag", [128, s // 128], DT, kind="ExternalInput")
    outT = nc.dram_tensor("outT", [F, n], DT, kind="ExternalOutput")
    chk = nc.dram_tensor("chk", [F, 512], DT, kind="ExternalOutput")

    RB = s // 512      # yT 512-col chunks (psum banks live in mm1)
    NG = n // 2048     # output column groups for mm2
    FTC = 4096         # featT chunk width

    with tile.TileContext(nc) as tc:
        with (
            tc.tile_pool(name="const", bufs=1) as constp,
            tc.tile_pool(name="xsb", bufs=1) as xsbp,
            tc.tile_pool(name="ysb", bufs=1) as ysbp,
            tc.tile_pool(name="ft", bufs=2) as ftp,
            tc.tile_pool(name="wt8", bufs=6) as wtp8,
            tc.tile_pool(name="wt16", bufs=3) as wtp16,
            tc.tile_pool(name="mt8", bufs=6) as mtp8,
            tc.tile_pool(name="mt16", bufs=3) as mtp16,
            tc.tile_pool(name="ot", bufs=2) as otp,
            tc.tile_pool(name="obs", bufs=1, space="PSUM") as obsp,
        ):
            w_sb = constp.tile([F, F], DT)
            nc.gpsimd.dma_start(w_sb[:], w[:])
            diag_sb = constp.tile([128, s // 128], DT)
            nc.gpsimd.dma_start(diag_sb[:], diag[:])
            id_sb = constp.tile([F, F], DT)
            make_identity(nc, id_sb[:])
            # bf16 copy of W for mm0 (moving operand)
            w16_sb = constp.tile([F, F], DTB)
            nc.vector.tensor_copy(w16_sb[:], w_sb[:])
            # DVE observer: one DVE op sees the diag DMA so later
            # tensor_scalar_muls only wait on their PE transpose.
            dvescr = constp.tile([128, s // 128], DT)
            nc.vector.tensor_copy(dvescr[:], diag_sb[:])

            # scratch PSUM bank for observer matmuls outside mm2 (one 32-col
            # slice each so nothing is ever dead-stored).
            obs_ps = obsp.tile([F, 512], DT)
            obs_n = [0]
            last_ob = [None]

            def observe(ap, target=None):
                """PE matmul reading `ap` ([P,32] or [32,32] slice): advances
                the PE clock past ap's producer with a single wait. `target`
                overrides the PSUM destination slice (mm2 writes into the
                idle bank set instead of obs_ps)."""
                if target is None:
                    target = obs_ps[:, (obs_n[0] % 16) * F:(obs_n[0] % 16 + 1) * F]
                obs_n[0] += 1
                ob = nc.tensor.matmul(target, ap, ap, start=True, stop=True)
                last_ob[0] = ob
                return ob

            def order_after_ob(mm):
                """Force the scheduler to keep `mm` after the latest observer
                so cross-engine waits land on the observer, keeping `mm` at a
                single sync wait."""
                if last_ob[0] is not None:
                    add_dep_helper(mm.ins, last_ob[0].ins, sync=False,
                                   reason="order after observer")

            def evacuate(dst, src, eng):
                """PSUM -> SBUF copy on DVE (eng 0) or Activation (eng 1)."""
                if eng == 0:
                    nc.vector.tensor_copy(dst, src)
                else:
                    nc.scalar.activation(dst, src,
                                         mybir.ActivationFunctionType.Copy)

            # packed layouts (see module docstring)
            x_sb = xsbp.tile([128, (n // 128) * F], DT16)   # [128, 4096]
            yT_sb = ysbp.tile([F, s], DT)                   # y.T, [32, 2048]
            y_sb = ysbp.tile([128, (s // 128) * F], DT16)   # [128, 512]

            observe(w_sb[:])
            observe(id_sb[:])
            if "mm0" in skip:
                nc.vector.memset(x_sb[:], 0.0)
                observe(x_sb[:, 0:F])
            if "tr" in skip:
                nc.vector.memset(y_sb[:], 0.0)
                observe(y_sb[:, 0:F])
            if "mm1" in skip:
                nc.vector.memset(yT_sb[:], 0.0)
                observe(yT_sb[:, 0:F])

            for _rep in range(reps):
                # ---- mm0: x = features @ W, written in packed row order via
                # strided stationary slices (stride 4 in the fp8 row range,
                # stride 2 in the fp16 range).
                with tc.tile_pool(name="ps_x", bufs=2, space="PSUM") as ps_x:
                  if "mm0" not in skip:
                    for fb in range(n // FTC):
                        ft = ftp.tile([F, FTC], DTB, tag="ft")
                        nc.sync.dma_start(ft[:], featT[:, fb * FTC:(fb + 1) * FTC])
                        r0 = fb * FTC  # first feature row of this ft tile
                        # packed (source AP, x_sb column group) pairs in tile
                        pl = []
                        if r0 < R81:
                            ftv = ft[:].rearrange("f (c p t) -> f c t p",
                                                  t=4, p=128)
                            for c in range(FTC // 512):
                                cc = (r0 // 512) + c
                                for t in range(4):
                                    pl.append((ftv[:, c, t, :], cc * 4 + t))
                        else:
                            ftv = ft[:].rearrange("f (c p t) -> f c t p",
                                                  t=2, p=128)
                            for c in range(FTC // 256):
                                cc = ((r0 - R81) // 256) + c
                                for t in range(2):
                                    pl.append((ftv[:, c, t, :],
                                               X8CH * 4 + cc * 2 + t))
                        for i, (src_ap, j) in enumerate(pl):
                            ps = ps_x.tile([128, F], DT)
                            mm = nc.tensor.matmul(ps[:], src_ap, w16_sb[:],
                                                  start=True, stop=True)
                            if i == 0:
                                order_after_ob(mm)
                            evacuate(x_sb[:, j * F:(j + 1) * F], ps[:], i % 2)
                        # PE sees this tile's DVE+Act evacuations so the next
                        # tile's matmuls only wait on their featT DMA.
                        jl2, jl = pl[-2][1], pl[-1][1]
                        observe(x_sb[:, jl2 * F:(jl2 + 1) * F])
                        observe(x_sb[:, jl * F:(jl + 1) * F])

                # ---- mm1: yT = x.T @ winvT  ([32, s] accumulated over 128 steps)
                with tc.tile_pool(name="ps_y", bufs=RB, space="PSUM") as ps_y:
                  if "mm1" not in skip:
                    yps = [ps_y.tile([F, 512], DT, name="yps", tag="yps")
                           for _ in range(RB)]
                    last_cl = None
                    for rb in range(RB):
                        # bank-claim: absorbs the PSUM bank-transition wait so the
                        # first accumulating matmul only waits on its DMA
                        cl = nc.tensor.matmul(yps[rb][:, 0:F], w_sb[:], w_sb[:],
                                              start=True, stop=True)
                        order_after_ob(cl)
                        last_cl = cl
                    nsteps = X8CH * 4 + X16CH * 2  # 128 accumulation steps
                    ki = 0
                    for cc in range(X8CH):  # fp8: 512-row packed chunks
                        wt = wtp8.tile([128, 4, s], DT8, tag="wt8")
                        nc.sync.dma_start(
                            wt[:], winvT8[cc * 512:(cc + 1) * 512, :].rearrange(
                                "(p t) r -> p t r", p=128))
                        for t in range(4):
                            j = cc * 4 + t
                            for rb in range(RB):
                                mm = nc.tensor.matmul(
                                    yps[rb][:],
                                    x_sb[:, j * F:(j + 1) * F],
                                    wt[:, t, rb * 512:(rb + 1) * 512],
                                    start=(ki == 0), stop=(ki == nsteps - 1),
                                )
                                if ki == 0 and rb == 0:
                                    add_dep_helper(mm.ins, last_cl.ins, sync=False,
                                                   reason="order after bank claims")
                            ki += 1
                    for cc in range(X16CH):  # fp16: 256-row packed chunks
                        wt = wtp16.tile([128, 2, s], DT16, tag="wt16")
                        nc.sync.dma_start(
                            wt[:], winvT16[cc * 256:(cc + 1) * 256, :].rearrange(
                                "(p t) r -> p t r", p=128))
                        for t in range(2):
                            j = X8CH * 4 + cc * 2 + t
                            for rb in range(RB):
                                nc.tensor.matmul(
                                    yps[rb][:],
                                    x_sb[:, j * F:(j + 1) * F],
                                    wt[:, t, rb * 512:(rb + 1) * 512],
                                    start=(ki == 0), stop=(ki == nsteps - 1),
                                )
                            ki += 1
                    for rb in range(RB):
                        evacuate(yT_sb[:, rb * 512:(rb + 1) * 512], yps[rb][:],
                                 rb % 2)

                # ---- transpose yT -> packed y tiles [128, 32], scaled by diag.
                # fp8 range (rows < K81): stride-4 column slices; fp16 range:
                # stride-2. diag_sb is host-packed in the same column order.
                with tc.tile_pool(name="ps_t", bufs=2, space="PSUM") as ps_t:
                  if "tr" not in skip:
                    observe(yT_sb[:, 2 * 512:2 * 512 + F])
                    observe(yT_sb[:, s - F:s])
                    pts = [ps_t.tile([128, F], DT, name="pt", tag="pt")
                           for _ in range(2)]
                    for i, pt in enumerate(pts):
                        cl = nc.tensor.matmul(pt[0:F, 0:F], w_sb[:], w_sb[:],
                                              start=True, stop=True)
                        order_after_ob(cl)
                    yTv8 = yT_sb[:, 0:K81].rearrange("f (c p t) -> f c t p",
                                                     t=4, p=128)
                    yTv16 = yT_sb[:, K81:s].rearrange("f (c p t) -> f c t p",
                                                      t=2, p=128)
                    ji = 0
                    for c in range(Y8CH):
                        for t in range(4):
                            pt = pts[ji % 2]
                            nc.tensor.transpose(pt[:], yTv8[:, c, t, :], id_sb[:])
                            dst = y_sb[:, ji * F:(ji + 1) * F]
                            if ji % 2 == 0:
                                nc.vector.tensor_scalar_mul(
                                    dst, pt[:], diag_sb[:, ji:ji + 1])
                            else:
                                nc.scalar.activation(
                                    dst, pt[:],
                                    mybir.ActivationFunctionType.Copy,
                                    scale=diag_sb[:, ji:ji + 1])
                            ji += 1
                    for c in range(Y16CH):
                        for t in range(2):
                            pt = pts[ji % 2]
                            nc.tensor.transpose(pt[:], yTv16[:, c, t, :], id_sb[:])
                            dst = y_sb[:, ji * F:(ji + 1) * F]
                            if ji % 2 == 0:
                                nc.vector.tensor_scalar_mul(
                                    dst, pt[:], diag_sb[:, ji:ji + 1])
                            else:
                                nc.scalar.activation(
                                    dst, pt[:],
                                    mybir.ActivationFunctionType.Copy,
                                    scale=diag_sb[:, ji:ji + 1])
                            ji += 1
                    observe(y_sb[:, (ji - 2) * F:(ji - 1) * F])
                    observe(y_sb[:, (ji - 1) * F:ji * F])

                # ---- mm2: outT = y'.T @ wvT  ([32, n] in groups of 2048 cols)
                # Groups alternate between two PSUM bank sets so evacuation of
                # group g overlaps accumulation of g+1; inter-group observers
                # write into the idle set, absorbing its WAR wait early.
                with tc.tile_pool(name="ps_o", bufs=7, space="PSUM") as ps_o:
                  if "mm2" not in skip:
                    # 2 alternating bank sets; obs_ps (idle during mm2: all
                    # observers here retarget) doubles as set B's 4th bank.
                    osets = [[ps_o.tile([F, 512], DT, name="ops", tag="ops")
                              for _ in range(4)],
                             [ps_o.tile([F, 512], DT, name="ops", tag="ops")
                              for _ in range(3)] + [obs_ps]]
                    for ng in range(NG):
                        ops = osets[ng % 2]
                        last_cl = None
                        for nb in range(4):
                            cl = nc.tensor.matmul(ops[nb][:, 0:F], w_sb[:], w_sb[:],
                                                  start=True, stop=True)
                            order_after_ob(cl)
                            last_cl = cl
                        ki = 0  # contraction step (of 16) within the group
                        for c in range(Y8CH):  # fp8: 512-row packed chunks
                            mt = mtp8.tile([128, 4, 2048], DT8, tag="mt8")
                            nc.sync.dma_start(
                                mt[:], wvT8[c * 512:(c + 1) * 512,
                                            ng * 2048:(ng + 1) * 2048].rearrange(
                                    "(p t) r -> p t r", p=128))
                            for t in range(4):
                                j = c * 4 + t
                                for nb in range(4):
                                    mm = nc.tensor.matmul(
                                        ops[nb][:],
                                        y_sb[:, j * F:(j + 1) * F],
                                        mt[:, t, nb * 512:(nb + 1) * 512],
                                        start=(ki == 0), stop=(ki == 15),
                                    )
                                    if ki == 0 and nb == 0:
                                        add_dep_helper(
                                            mm.ins, last_cl.ins, sync=False,
                                            reason="order after bank claims")
                                ki += 1
                        for c in range(Y16CH):  # fp16: 256-row packed chunks
                            mt = mtp16.tile([128, 2, 2048], DT16, tag="mt16")
                            nc.sync.dma_start(
                                mt[:], wvT16[c * 256:(c + 1) * 256,
                                             ng * 2048:(ng + 1) * 2048].rearrange(
                                    "(p t) r -> p t r", p=128))
                            for t in range(2):
                                j = Y8CH * 4 + c * 2 + t
                                for nb in range(4):
                                    nc.tensor.matmul(
                                        ops[nb][:],
                                        y_sb[:, j * F:(j + 1) * F],
                                        mt[:, t, nb * 512:(nb + 1) * 512],
                                        start=(ki == 0), stop=(ki == 15),
                                    )
                                ki += 1
                        ot = otp.tile([F, 2048], DT, tag="ot")
                        for nb in range(4):
                            evacuate(ot[:, nb * 512:(nb + 1) * 512], ops[nb][:],
                                     nb % 2)
                        nc.sync.dma_start(outT[:, ng * 2048:(ng + 1) * 2048], ot[:])
                        # PE sees this group's DVE+Act evacuations before the
                        # bank set comes back; write into the idle set's first
                        # tile (real data, reset by its next start=True).
                        nxt = osets[(ng + 1) % 2][0]
                        observe(ot[:, 2 * 512:2 * 512 + F], target=nxt[:, 0:F])
                        observe(ot[:, 3 * 512:3 * 512 + F], target=nxt[:, F:2 * F])

            chk_sb = constp.tile([F, 512], DT)
            nc.vector.tensor_copy(chk_sb[:], obs_ps[:])
            nc.gpsimd.dma_start(chk[:], chk_sb[:])

    _split_excess_waits(nc)
    return nc


def _split_excess_waits(nc, limit=1):
    """Walrus allows a single sync-wait slot on fused fp32 matmuls and DMA
    triggers. Move any extra waits onto standalone EventSemaphore
    instructions inserted just before the offender in its engine stream
    (what raw-bass wait_ge would emit)."""
    nev = [0]
    for f in nc.m.functions:
        for b in f.blocks:
            out = []
            changed = False
            for inst in b.instructions:
                si = inst.sync_info
                waits = list(si.on_wait) if si is not None else []
                if len(waits) > limit:
                    changed = True
                    for wv in waits[:-limit]:
                        ev = mybir.InstEventSemaphore(
                            name=f"splitwait_{nev[0]}", engine=inst.engine,
                            ins=[], outs=[])
                        nev[0] += 1
                        ev.sync_info = mybir.SyncInfo(on_wait=[wv], on_update=[])
                        out.append(ev)
                    inst.sync_info = mybir.SyncInfo(
                        on_wait=waits[-limit:], on_update=list(si.on_update))
                out.append(inst)
            if changed:
                b.instructions = out
    return nc


def _cast_T(a, np_dt):
    """Cache-blocked cast + out-of-place transpose ([r,c] fp32 -> [c,r] np_dt)."""
    r, c = a.shape
    out = np.empty((c, r), dtype=np_dt)
    B = 512
    for i in range(0, r, B):
        for k in range(0, c, B):
            out[k:k + B, i:i + B] = a[i:i + B, k:k + B].astype(np_dt).T
    return out


def _pack_diag(d):
    """Host-pack the per-core diag slice [S] into the y_sb column order:
    fp8 range: col j = c*4+t -> row c*512 + 4p + t (c < Y8CH);
    fp16 range: col Y8CH*4 + c*2+t -> row K81 + c*256 + 2p + t."""
    out = np.empty((128, S // 128), dtype=np.float32)
    p = np.arange(128)
    j = 0
    for c in range(Y8CH):
        for t in range(4):
            out[:, j] = d[c * 512 + 4 * p + t]
            j += 1
    for c in range(Y16CH):
        for t in range(2):
            out[:, j] = d[K81 + c * 256 + 2 * p + t]
            j += 1
    return np.ascontiguousarray(out)


def _shard_inputs(features, wavelets, wavelets_inv, diag_filter, weight_matrix):
    from concurrent.futures import ThreadPoolExecutor
    featT = np.ascontiguousarray(features.T.astype(NPB))
    with ThreadPoolExecutor(max_workers=16) as ex:
        wvT8_parts = list(ex.map(
            lambda j: _cast_T(wavelets[:, j * S:j * S + K81], NP8),
            range(NCORES)))
        wvT16_parts = list(ex.map(
            lambda j: _cast_T(wavelets[:, j * S + K81:(j + 1) * S], NP16),
            range(NCORES)))
        winvT8_parts = list(ex.map(
            lambda j: _cast_T(wavelets_inv[j * S:(j + 1) * S, :R81], NP8),
            range(NCORES)))
        winvT16_parts = list(ex.map(
            lambda j: _cast_T(wavelets_inv[j * S:(j + 1) * S, R81:], NP16),
            range(NCORES)))
    in_maps = []
    for j in range(NCORES):
        r0, r1 = j * S, (j + 1) * S
        in_maps.append({
            "featT": featT,
            "w": np.ascontiguousarray(weight_matrix),
            "winvT8": winvT8_parts[j],
            "winvT16": winvT16_parts[j],
            "wvT8": wvT8_parts[j],
            "wvT16": wvT16_parts[j],
            "diag": _pack_diag(diag_filter[r0:r1]),
        })
    return in_maps


def _run(inputs, trace=False, **trace_kwargs):
    in_maps = _shard_inputs(
        np.asarray(inputs["features"], dtype=np.float32),
        np.asarray(inputs["wavelets"], dtype=np.float32),
        np.asarray(inputs["wavelets_inv"], dtype=np.float32),
        np.asarray(inputs["diag_filter"], dtype=np.float32),
        np.asarray(inputs["weight_matrix"], dtype=np.float32),
    )
    nc = build_bass()
    res = run_bass_kernel_spmd(nc, in_maps, list(range(NCORES)), trace=trace,
                               **trace_kwargs)
    acc = np.zeros((F, N), dtype=np.float64)
    for j in range(NCORES):
        acc += res.results[j]["outT"]
    out = np.ascontiguousarray(acc.T.astype(np.float32))
    return out, res


def kernel(**inputs):
    out, _ = _run(inputs, trace=False)
    return out


def kernel_traced(**inputs):
    out, res = _run(inputs, trace=True)
    return out, res
